# revision 1
# baseline (speedup 1.0000x reference)
"""Trainium2 Bass kernel for nn_RRE_GNN_raw (GNN message passing).

Strategy: sort edges by destination node (obj) on the host, shard NODES
across the 8 cores (each core owns 49 node-tiles of 128 nodes and all
edges pointing into them -> no collectives needed). Per 128-edge chunk the
device:
  - gathers hidden[sub] rows via indirect DMA (f16),
  - gathers h_r = rela[rel] and h_qr = rela[q_rel[r_idx]] ON-CHIP via
    one-hot matmuls against the SBUF-resident 401-row rela table
    (indices composed on the host; one-hot built from a DMA-broadcast
    index row + DVE compares),
  - runs the GRU gate + attention in feature-major f16 matmuls,
  - reduces the softmax-weighted segment sums with scaled one-hot
    matmuls accumulated in PSUM per node-tile.
"""
import sys

sys.path.insert(0, '/opt/trn_rl_repo')

import json
import numpy as np

import concourse.bass as bass
import concourse.tile as tile
from concourse import mybir
from concourse.bass_utils import run_bass_kernel_spmd
from concourse.vector_clock import ScopedClock
import bass_rust

# ---------------------------------------------------------------- constants
P = 128            # partitions / tile edge
D = 128            # feature dim
A = 128            # attention dim
N_NODE = 50000
E_EDGE = 600000
NQ = 1024
NRE = 401
NREP = 512         # rela table padded to 4 chunks of 128 rows
NCORES = 8
T_TILES = 49       # node tiles per core
NODES_PER_CORE = T_TILES * P          # 6272
N_PAD = NCORES * NODES_PER_CORE       # 50176

f16 = mybir.dt.float16
f32 = mybir.dt.float32
i32 = mybir.dt.int32

AF = mybir.ActivationFunctionType
ALU = mybir.AluOpType


# ------------------------------------------------- harness compatibility fixes
class _TC(tile.TileContext):
    """TileContext whose kernel-tail drain emits one wait per instruction
    (the walrus build here rejects instructions with >1 inline sync wait)."""

    def _drain_and_barrier(self, tick_clock, wait_clock):
        nc = self.nc
        probe = nc.sync.nop(nofuse=True)
        wait_clock.add_sem_waits(probe.ins,
                                 ScopedClock({None: tick_clock.global_clock}))
        waits = list(probe.ins.sync_info.on_wait)
        probe.ins.sync_info = bass_rust.SyncInfo(on_wait=[], on_update=[])
        name2sem = {s.name: s for s in self.sems.allocated().values()}
        for w in waits:
            nc.sync.wait_ge(name2sem[w.ant_name], w.wait_value)
        nc.sync.drain()
        nc.all_engine_barrier()
        popped = nc._tile_sem_poison_stack.pop()
        assert popped is self._sem_poison
        nc.clear_and_free_semaphores(list(self.sems.allocated().values()))
        nc.all_engine_barrier()


def _split_bir_waits(bir_json: bytes) -> bytes:
    """Hoist all-but-one sync wait of any instruction onto standalone
    EventSemaphore ops placed just before it on the same engine queue."""
    d = json.loads(bir_json)
    changed = False
    for func in d.get("functions", []):
        for blk in func.get("blocks", []):
            out = []
            for inst in blk["instructions"]:
                si = inst.get("sync_info")
                waits = si.get("on_wait", []) if si else []
                if len(waits) > 1:
                    for k, w in enumerate(waits[:-1]):
                        out.append({
                            "name": f"{inst['name']}-hw{k}",
                            "opcode": "EventSemaphore",
                            "engine": inst["engine"],
                            "ins": [], "outs": [],
                            "sync_info": {"on_update": [], "on_wait": [w]},
                        })
                    si["on_wait"] = waits[-1:]
                    changed = True
                out.append(inst)
            blk["instructions"] = out
    if not changed:
        return bir_json
    return json.dumps(d).encode()


_hook_installed = False


def _install_wait_splitter():
    global _hook_installed
    if _hook_installed:
        return
    import concourse.bass2jax as bass2jax
    orig = bass2jax.compile_bir_kernel

    def patched(bir_json, tmpdir, neff_name="file.neff"):
        return orig(_split_bir_waits(bir_json), tmpdir, neff_name=neff_name)

    bass2jax.compile_bir_kernel = patched
    _hook_installed = True


# ---------------------------------------------------------------- host prep
def _host_prep(hidden, rela_embed, q_rel, edges):
    """Sort/shard/pad on the host. Returns per-core metadata arrays and the
    static per-tile chunk counts (shared by all cores -> one SPMD program)."""
    r_idx = edges[:, 0].astype(np.int64)
    rel = edges[:, 2].astype(np.int64)
    sub = edges[:, 4].astype(np.int64)
    obj = edges[:, 5].astype(np.int64)
    q_rel = np.asarray(q_rel, np.int64)

    order = np.argsort(obj, kind="stable")
    obj_s = obj[order]
    sub_s = sub[order]
    rel_s = rel[order]
    qc_s = q_rel[r_idx[order]]        # composed: h_qr = rela[q_rel[r_idx]]

    # node_group: last write in ORIGINAL edge order (matches reference)
    node_group = np.zeros(N_PAD, np.int64)
    node_group[obj] = r_idx

    counts = np.bincount(obj_s, minlength=N_PAD)
    starts = np.zeros(N_PAD + 1, np.int64)
    np.cumsum(counts, out=starts[1:])

    n_gtiles = NCORES * T_TILES
    gc = np.zeros(n_gtiles, np.int64)
    for g in range(n_gtiles):
        gc[g] = starts[min((g + 1) * P, N_PAD)] - starts[g * P]
    chunks = (gc + P - 1) // P
    C_list = [max(1, int(chunks[t::T_TILES].max())) for t in range(T_TILES)]
    col_off = np.zeros(T_TILES + 1, np.int64)
    np.cumsum(C_list, out=col_off[1:])
    CT = int(col_off[-1])

    off_sub = np.zeros((NCORES, P, CT), np.int32)
    relqc = np.zeros((NCORES, CT, 2 * P), np.float16)
    obj_f = np.full((NCORES, P, CT), -1.0, np.float32)

    for core in range(NCORES):
        for t in range(T_TILES):
            g = core * T_TILES + t
            lo = starts[g * P]
            hi = starts[min((g + 1) * P, N_PAD)]
            L = int(hi - lo)
            slot = np.arange(L)
            pp = slot % P
            cc = col_off[t] + slot // P
            off_sub[core, pp, cc] = sub_s[lo:hi]
            relqc[core, cc, pp] = rel_s[lo:hi].astype(np.float16)
            relqc[core, cc, P + pp] = qc_s[lo:hi].astype(np.float16)
            obj_f[core, pp, cc] = (obj_s[lo:hi] - g * P).astype(np.float32)

    ng_off = node_group.reshape(NCORES, T_TILES, P).transpose(0, 2, 1) \
                       .astype(np.int32).copy()    # [core, P, T]

    return dict(
        C_list=C_list, col_off=col_off, CT=CT,
        off_sub=off_sub, relqc=relqc, obj_f=obj_f, ng_off=ng_off,
    )


# ------------------------------------------------------------ device program
def _build_program(C_list, col_off, CT):
    nc = bass.Bass()
    dp = nc.declare_dram_parameter

    hid16 = dp("hid16", [N_NODE, D], f16, isOutput=False)
    relap = dp("relap", [NREP, D], f16, isOutput=False)   # padded rela table
    hq32 = dp("hq32", [NQ, D], f32, isOutput=False)

    wz_t = dp("wz_t", [D, D], f16, isOutput=False)
    wz_b = dp("wz_b", [D, D], f16, isOutput=False)
    uz = dp("uz", [D, D], f16, isOutput=False)
    wr_t = dp("wr_t", [D, D], f16, isOutput=False)
    wr_b = dp("wr_b", [D, D], f16, isOutput=False)
    ur = dp("ur", [D, D], f16, isOutput=False)
    wh_t = dp("wh_t", [D, D], f16, isOutput=False)
    wh_b = dp("wh_b", [D, D], f16, isOutput=False)
    uh = dp("uh", [D, D], f16, isOutput=False)
    ws = dp("ws", [D, A], f16, isOutput=False)
    wqr = dp("wqr", [D, A], f16, isOutput=False)
    walpha = dp("walpha", [A, 1], f16, isOutput=False)
    wh_out = dp("wh_out", [D, D], f16, isOutput=False)
    bz = dp("bz", [D, 1], f32, isOutput=False)
    br = dp("br", [D, 1], f32, isOutput=False)
    bh = dp("bh", [D, 1], f32, isOutput=False)
    bqr = dp("bqr", [A, 1], f32, isOutput=False)
    balpha = dp("balpha", [P, 1], f32, isOutput=False)
    iota_d = dp("iota", [P, P], f32, isOutput=False)
    iosh_d = dp("iosh", [P, 4], f32, isOutput=False)      # iota + 128k
    ident_d = dp("ident", [P, P], f32, isOutput=False)

    off_sub_d = dp("off_sub", [P, CT], i32, isOutput=False)
    relqc_d = dp("relqc", [CT, 2 * P], f16, isOutput=False)
    obj_f_d = dp("obj_f", [P, CT], f32, isOutput=False)
    ng_off_d = dp("ng_off", [P, T_TILES], i32, isOutput=False)

    out_ht = dp("out_ht", [P, T_TILES * P], f32, isOutput=True)
    out_hnqr = dp("out_hnqr", [T_TILES * P, D], f32, isOutput=True)

    Cmax = max(C_list)

    from contextlib import ExitStack
    with _TC(nc) as tc, ExitStack() as ctx:
        const = ctx.enter_context(tc.tile_pool(name="const", bufs=1))
        meta = ctx.enter_context(tc.tile_pool(name="meta", bufs=1))
        gat = ctx.enter_context(tc.tile_pool(name="gat", bufs=4))
        trn = ctx.enter_context(tc.tile_pool(name="trn", bufs=6))
        ohp = ctx.enter_context(tc.tile_pool(name="ohp", bufs=3))
        bcp = ctx.enter_context(tc.tile_pool(name="bcp", bufs=4))
        ck = ctx.enter_context(tc.tile_pool(name="ck", bufs=7))
        pwp = ctx.enter_context(tc.tile_pool(name="pwp", bufs=24))
        fin = ctx.enter_context(tc.tile_pool(name="fin", bufs=2))
        # PSUM banks: shared chunk bank (hrq->gates->transpose) x5,
        # agg(+acols) x3 = 8 of 8
        psg = ctx.enter_context(tc.tile_pool(name="psg", bufs=5, space="PSUM"))
        psa = ctx.enter_context(tc.tile_pool(name="psa", bufs=3, space="PSUM"))

        def load(pool, dram_t, shape, dt, tag):
            t = pool.tile(shape, dt, tag=tag)
            nc.sync.dma_start(t[:], dram_t[:])
            return t

        wz_t_s = load(const, wz_t, [D, D], f16, "wz_t")
        wz_b_s = load(const, wz_b, [D, D], f16, "wz_b")
        uz_s = load(const, uz, [D, D], f16, "uz")
        wr_t_s = load(const, wr_t, [D, D], f16, "wr_t")
        wr_b_s = load(const, wr_b, [D, D], f16, "wr_b")
        ur_s = load(const, ur, [D, D], f16, "ur")
        wh_t_s = load(const, wh_t, [D, D], f16, "wh_t")
        wh_b_s = load(const, wh_b, [D, D], f16, "wh_b")
        uh_s = load(const, uh, [D, D], f16, "uh")
        ws_s = load(const, ws, [D, A], f16, "ws")
        wqr_s = load(const, wqr, [D, A], f16, "wqr")
        walpha_s = load(const, walpha, [A, 1], f16, "walpha")
        whout_s = load(const, wh_out, [D, D], f16, "whout")
        bz_s = load(const, bz, [D, 1], f32, "bz")
        br_s = load(const, br, [D, 1], f32, "br")
        bh_s = load(const, bh, [D, 1], f32, "bh")
        bqr_s = load(const, bqr, [A, 1], f32, "bqr")
        balpha_s = load(const, balpha, [P, 1], f32, "balpha")
        iota_s = load(const, iota_d, [P, P], f32, "iota")
        iosh_s = load(const, iosh_d, [P, 4], f32, "iosh")
        ident_s = load(const, ident_d, [P, P], f32, "ident")
        ones_s = const.tile([P, 1], f16, tag="ones")
        nc.vector.memset(ones_s[:], 1.0)
        # rela table as 4 lhsT chunks [128 rows, D]
        rela_s = const.tile([P, 4 * D], f16, tag="rela")
        for k in range(4):
            nc.sync.dma_start(rela_s[:, k * D:(k + 1) * D],
                              relap[k * P:(k + 1) * P, :])

        off_sub_s = load(meta, off_sub_d, [P, CT], i32, "off_sub")
        obj_f_s = load(meta, obj_f_d, [P, CT], f32, "obj_f")
        ng_off_s = load(meta, ng_off_d, [P, T_TILES], i32, "ng_off")

        mm = nc.tensor.matmul
        act = nc.scalar.activation

        CW = 2 * P
        PF = 3                     # tiles of gather/bcast prefetch depth

        def emit_fetch(t):
            Ct = C_list[t]
            co = int(col_off[t])
            Et = Ct * P
            hs_raw = gat.tile([P, Cmax * P], f16, tag="hs_raw")
            for c in range(Ct):
                nc.gpsimd.indirect_dma_start(
                    out=hs_raw[:, c * P:(c + 1) * P], out_offset=None,
                    in_=hid16[:],
                    in_offset=bass.IndirectOffsetOnAxis(
                        ap=off_sub_s[:, co + c:co + c + 1], axis=0))
            hsT = trn.tile([P, Cmax * P], f16, tag="hsT")
            nc.sync.dma_start_transpose(
                out=hsT[:, :Et].rearrange("p (k d) -> p k d", k=Ct),
                in_=hs_raw[:, :Et])
            bct = bcp.tile([P, Cmax * CW], f16, tag="bct")
            nc.sync.dma_start(
                bct[:, :Ct * CW],
                relqc_d[co:co + Ct, :].rearrange(
                    "a b -> (a b)")[None, :].to_broadcast([P, Ct * CW]))
            return hsT, bct

        fetched = {t: emit_fetch(t) for t in range(min(PF, T_TILES))}

        def tile_gen(t):
            Ct = C_list[t]
            co = int(col_off[t])
            Et = Ct * P
            if t + PF < T_TILES:
                fetched[t + PF] = emit_fetch(t + PF)
            hsT, bct = fetched.pop(t)

            agg = psa.tile([P, 160], f32, tag="agg")
            # one dummy matmul clears the whole agg bank (start=True); all
            # later matmuls into it use start=False on fresh has_written bits
            mm(agg[:, 156:157], lhsT=wz_t_s[:], rhs=ones_s[:],
               start=True, stop=True, skip_group_check=True)

            # build all one-hots for this tile in 4 batched compares
            oh_t = ohp.tile([P, Cmax * 4 * CW], f16, tag="oh_t")
            for k in range(4):
                nc.vector.tensor_scalar(
                    out=oh_t[:, k * Ct * CW:(k + 1) * Ct * CW],
                    in0=bct[:, :Ct * CW],
                    scalar1=iosh_s[:, k:k + 1], scalar2=None,
                    op0=ALU.is_equal)

            pend = []      # agg matmuls of the previous macro
            curr = []      # (pw, cl) of the current macro
            MAC = 4
            msgT4 = None
            for c in range(Ct):
                sl = slice(c * P, (c + 1) * P)
                cl = c % MAC
                if cl == 0:
                    msgT4 = ck.tile([P, MAC * P], f32, tag="msgT4")

                # --- on-chip gather of h_r^T | h_qr^T via one-hot
                # matmuls, into the shared chunk bank (cleared at k==0)
                gb = psg.tile([P, 512], f32, tag="gates")
                for k in range(4):
                    ob = (k * Ct + c) * CW
                    mm(gb[:, 0:2 * P], lhsT=rela_s[:, k * D:(k + 1) * D],
                       rhs=oh_t[:, ob:ob + CW],
                       start=(k == 0), stop=(k == 3))
                hrq_sb = ck.tile([P, 2 * P], f16, tag="hrqsb")
                nc.vector.tensor_copy(hrq_sb[:], gb[:, 0:2 * P])
                hrT = hrq_sb[:, 0:P]
                hqrT = hrq_sb[:, P:2 * P]

                # --- GRU gates reuse the bank (start=True reclears)
                zp = gb[:, 0:128]
                rp = gb[:, 128:256]
                hp = gb[:, 256:384]
                apre = gb[:, 384:512]

                mm(zp, lhsT=wz_t_s[:], rhs=hrT, start=True, stop=False)
                mm(zp, lhsT=wz_b_s[:], rhs=hqrT, start=False, stop=False)
                mm(zp, lhsT=uz_s[:], rhs=hsT[:, sl], start=False, stop=True,
                   skip_group_check=True)
                mm(rp, lhsT=wr_t_s[:], rhs=hrT, start=False, stop=False,
                   skip_group_check=True)
                mm(rp, lhsT=wr_b_s[:], rhs=hqrT, start=False, stop=False)
                mm(rp, lhsT=ur_s[:], rhs=hsT[:, sl], start=False, stop=True,
                   skip_group_check=True)
                mm(hp, lhsT=wh_t_s[:], rhs=hrT, start=False, stop=False,
                   skip_group_check=True)
                mm(hp, lhsT=wh_b_s[:], rhs=hqrT, start=False, stop=False)

                z_sb = ck.tile([P, P], f16, tag="z")
                act(z_sb[:], zp, AF.Sigmoid, bias=bz_s[:, :1])
                r_sb = ck.tile([P, P], f16, tag="r")
                act(r_sb[:], rp, AF.Sigmoid, bias=br_s[:, :1])

                rh = ck.tile([P, P], f16, tag="rh")
                nc.vector.tensor_tensor(out=rh[:], in0=r_sb[:],
                                        in1=hsT[:, sl], op=ALU.mult)
                mm(hp, lhsT=uh_s[:], rhs=rh[:], start=False, stop=True,
                   skip_group_check=True)
                ht = ck.tile([P, P], f16, tag="ht")
                act(ht[:], hp, AF.Tanh, bias=bh_s[:, :1])

                # message^T = hsT + z*(ht - hsT)
                dd = ck.tile([P, P], f16, tag="dd")
                nc.vector.tensor_tensor(out=dd[:], in0=ht[:],
                                        in1=hsT[:, sl], op=ALU.subtract)
                zd = ck.tile([P, P], f16, tag="zd")
                nc.vector.tensor_tensor(out=zd[:], in0=z_sb[:],
                                        in1=dd[:], op=ALU.mult)
                msgT = msgT4[:, cl * P:(cl + 1) * P]
                nc.vector.tensor_tensor(out=msgT, in0=zd[:],
                                        in1=hsT[:, sl], op=ALU.add)

                # msg@Ws split as zd@Ws + hsT@Ws (PSUM accumulates):
                # avoids a DVE f32->f16 cast of the message and lets the
                # first matmul start before the f32 message exists
                mm(apre, lhsT=ws_s[:], rhs=zd[:], start=False,
                   stop=False, skip_group_check=True)
                mm(apre, lhsT=ws_s[:], rhs=hsT[:, sl], start=False,
                   stop=False, skip_group_check=True)
                mm(apre, lhsT=wqr_s[:], rhs=hqrT, start=False, stop=True)
                relu_sb = ck.tile([P, P], f16, tag="relu")
                act(relu_sb[:], apre, AF.Relu, bias=bqr_s[:, :1])

                # transpose message on PE into the (fully consumed)
                # gates bank; start=True reclears the bank
                # attention scalar + softmax weight stay in-line
                mm(agg[:, 140 + c:141 + c], lhsT=relu_sb[:], rhs=walpha_s[:],
                   start=False, stop=True, skip_group_check=True)
                expc = ck.tile([P, 1], f32, tag="expc")
                act(expc[:, :1], agg[:, 140 + c:141 + c], AF.Exp,
                    bias=balpha_s[:, :1])
                pw = pwp.tile([P, P], f16, tag="pw")
                nc.vector.tensor_scalar(
                    out=pw[:], in0=iota_s[:],
                    scalar1=obj_f_s[:, co + c:co + c + 1],
                    scalar2=expc[:, :1],
                    op0=ALU.is_equal, op1=ALU.mult)
                curr.append((pw, c, cl))

                # f32 PE transpose into the consumed gates bank, then
                # copy the edge-major message out to SBUF
                if cl == 0:
                    msgE4 = ck.tile([P, MAC * P], f16, tag="msgE4")
                mm(gb[:, 384:512], lhsT=msgT, rhs=ident_s[:],
                   is_transpose=True, start=True, stop=True,
                   skip_group_check=True)
                nc.vector.tensor_copy(msgE4[:, cl * P:(cl + 1) * P],
                                      gb[:, 384:512])

                if cl == MAC - 1 or c == Ct - 1:
                    # flush the PREVIOUS macro's agg matmuls; the current
                    # macro's are deferred so PE never waits on the
                    # transpose chain.
                    for pw_, c_, cl_, msgE_ in pend:
                        # start=True would clear the whole agg bank; only
                        # chunk 0's weighted-message matmul may use it.
                        mm(agg[:, 0:P], lhsT=pw_[:],
                           rhs=msgE_[:, cl_ * P:(cl_ + 1) * P],
                           start=False, stop=(c_ == Ct - 1),
                           skip_group_check=True)
                        mm(agg[:, P:P + 1], lhsT=pw_[:], rhs=ones_s[:],
                           start=False, stop=(c_ == Ct - 1),
                           skip_group_check=True)
                    pend = [(pw_, c_, cl_, msgE4) for (pw_, c_, cl_) in curr]
                    curr = []
                yield
            for pw_, c_, cl_, msgE_ in pend:
                mm(agg[:, 0:P], lhsT=pw_[:],
                   rhs=msgE_[:, cl_ * P:(cl_ + 1) * P],
                   start=False, stop=(c_ == Ct - 1),
                   skip_group_check=True)
                mm(agg[:, P:P + 1], lhsT=pw_[:], rhs=ones_s[:],
                   start=False, stop=(c_ == Ct - 1),
                   skip_group_check=True)

            # --- finalize node tile
            recip = fin.tile([P, 1], f32, tag="recip")
            nc.vector.reciprocal(recip[:], agg[:, P:P + 1])
            magg = fin.tile([P, P], f16, tag="magg")
            nc.vector.tensor_scalar(out=magg[:], in0=agg[:, 0:P],
                                    scalar1=recip[:, :1], scalar2=None,
                                    op0=ALU.mult)
            maggT = fin.tile([P, P], f16, tag="maggT")
            nc.sync.dma_start_transpose(out=maggT[:], in_=magg[:])
            hf = psg.tile([P, 512], f32, tag="gates")
            mm(hf[:, 0:P], lhsT=whout_s[:], rhs=maggT[:],
               start=True, stop=True)
            hnew = fin.tile([P, P], f32, tag="hnew")
            act(hnew[:], hf[:, 0:P], AF.Relu)
            nc.sync.dma_start(out_ht[:, t * P:(t + 1) * P], hnew[:])

            hnq = fin.tile([P, D], f32, tag="hnq")
            nc.gpsimd.indirect_dma_start(
                out=hnq[:], out_offset=None, in_=hq32[:],
                in_offset=bass.IndirectOffsetOnAxis(
                    ap=ng_off_s[:, t:t + 1], axis=0))
            nc.sync.dma_start(out_hnqr[t * P:(t + 1) * P, :], hnq[:])
            yield

        # drive pairs of tiles with interleaved chunk emission so each
        # engine's in-order queue always has an independent stream to run
        t = 0
        while t < T_TILES:
            gens = [tile_gen(x) for x in range(t, min(t + 3, T_TILES))]
            alive = list(gens)
            while alive:
                for g in list(alive):
                    try:
                        next(g)
                    except StopIteration:
                        alive.remove(g)
            t += 3

    return nc


# ----------------------------------------------------------------- kernel()
def kernel(hidden, rela_embed, Wz, Uz, bz, Wr_g, Ur, br, Whh, Uh, bh,
           Ws_attn, Wqr_attn, b_qr, w_alpha, b_alpha, W_h,
           q_rel, edges, n_node):
    _install_wait_splitter()

    hidden = np.asarray(hidden, np.float32)
    rela_embed = np.asarray(rela_embed, np.float32)
    edges = np.asarray(edges)
    q_rel = np.asarray(q_rel)

    meta = _host_prep(hidden, rela_embed, q_rel, edges)
    C_list, col_off, CT = meta["C_list"], meta["col_off"], meta["CT"]

    hq = rela_embed[np.asarray(q_rel, np.int64)]          # [NQ, D] f32
    relap = np.zeros((NREP, D), np.float32)
    relap[:NRE] = rela_embed

    nc = _build_program(C_list, col_off, CT)

    iosh = (np.arange(P, dtype=np.float32)[:, None]
            + 128.0 * np.arange(4, dtype=np.float32)[None, :]).copy()

    common = {
        "hid16": hidden.astype(np.float16),
        "relap": relap.astype(np.float16),
        "hq32": hq.astype(np.float32),
        "wz_t": np.asarray(Wz[:D], np.float16),
        "wz_b": np.asarray(Wz[D:], np.float16),
        "uz": np.asarray(Uz, np.float16),
        "wr_t": np.asarray(Wr_g[:D], np.float16),
        "wr_b": np.asarray(Wr_g[D:], np.float16),
        "ur": np.asarray(Ur, np.float16),
        "wh_t": np.asarray(Whh[:D], np.float16),
        "wh_b": np.asarray(Whh[D:], np.float16),
        "uh": np.asarray(Uh, np.float16),
        "ws": np.asarray(Ws_attn, np.float16),
        "wqr": np.asarray(Wqr_attn, np.float16),
        "walpha": np.asarray(w_alpha, np.float16).reshape(A, 1),
        "wh_out": np.asarray(W_h, np.float16),
        "bz": np.asarray(bz, np.float32).reshape(D, 1),
        "br": np.asarray(br, np.float32).reshape(D, 1),
        "bh": np.asarray(bh, np.float32).reshape(D, 1),
        "bqr": np.asarray(b_qr, np.float32).reshape(A, 1),
        "balpha": np.full((P, 1), float(np.asarray(b_alpha).reshape(-1)[0]),
                          np.float32),
        "iota": np.broadcast_to(np.arange(P, dtype=np.float32),
                                (P, P)).copy(),
        "iosh": iosh,
        "ident": np.eye(P, dtype=np.float32),
    }
    in_maps = []
    for core in range(NCORES):
        m = dict(common)
        m["off_sub"] = meta["off_sub"][core]
        m["relqc"] = meta["relqc"][core]
        m["obj_f"] = meta["obj_f"][core]
        m["ng_off"] = meta["ng_off"][core]
        in_maps.append(m)

    res = run_bass_kernel_spmd(nc, in_maps, list(range(NCORES))).results

    hidden_new = np.empty((N_PAD, D), np.float32)
    h_n_qr = np.empty((N_PAD, D), np.float32)
    for core in range(NCORES):
        lo = core * NODES_PER_CORE
        hi = lo + NODES_PER_CORE
        hidden_new[lo:hi] = res[core]["out_ht"].T
        h_n_qr[lo:hi] = res[core]["out_hnqr"]

    return hidden_new[:N_NODE], h_n_qr[:N_NODE]



# revision 17
# speedup vs baseline: 2.0547x; 2.0547x over previous
"""Trainium2 Bass kernel for nn_RRE_GNN_raw (GNN message passing), v3.

Strategy: sort edges by destination node (obj) on the host, shard NODES
across the 8 cores (each core owns 49 node-tiles of 128 nodes and all
edges pointing into them -> no collectives). All per-edge row gathers
use int16 DMAGather with transpose=True, which lands rows directly
FEATURE-major in SBUF:
  - hidden[sub] is gathered from two <=32768-row halves of the hidden
    table (int16 index limit); each tile's edge slots are grouped by
    half so every gather call covers one contiguous slot range,
  - h_r = rela[rel] and h_qr = rela[q_rel[r_idx]] come from the 401-row
    rela table in one call per slot section.
Compute runs in macros of up to 4x128 edges: 9 gate matmuls + 2
attention matmuls at macro width, activations/DVE ops at macro width,
message transposed back edge-major with f16 PE transposes, and the
softmax-weighted segment sums accumulate in PSUM per node-tile via
scaled one-hot matmuls (ones column folded into the 129-wide matmul).

Scheduling: ~6 macro "streams" advance round-robin one stage per sweep,
each stream owning ONE rotating PSUM bank (zp->rp->hp->apre->msgE
phases reuse it), so every engine's in-order queue always holds ready
work from other streams and head-of-line blocking is minimized.

The h_n_qr output is produced by batched DMAGather at kernel start.
"""
import sys

sys.path.insert(0, '/opt/trn_rl_repo')

import json
import numpy as np

import concourse.bass as bass
import concourse.tile as tile
from concourse import library_config
from concourse import mybir
from concourse.bass_utils import run_bass_kernel_spmd
from concourse.vector_clock import ScopedClock
import bass_rust

# ---------------------------------------------------------------- constants
P = 128            # partitions / tile edge
D = 128            # feature dim
A = 128            # attention dim
N_NODE = 50000
NSPLIT = 32768     # int16 index limit for dma_gather tables
NQ = 1024
NRE = 401
NREP = 512         # rela table padded
NCORES = 8
T_TILES = 49       # node tiles per core
NODES_PER_CORE = T_TILES * P          # 6272
N_PAD = NCORES * NODES_PER_CORE       # 50176
MACRO = 4          # chunks per macro (512 edges)
NSTREAM = 6        # concurrent macro streams (PSUM G banks)
PF_TILES = 3       # gather prefetch depth in tiles

f16 = mybir.dt.float16
f32 = mybir.dt.float32
i32 = mybir.dt.int32
i16 = mybir.dt.int16

DISABLE = set()          # debug bisection knobs
AF = mybir.ActivationFunctionType
ALU = mybir.AluOpType


# ------------------------------------------------- harness compatibility fixes
class _TC(tile.TileContext):
    """TileContext whose kernel-tail drain emits one wait per instruction
    (the walrus build here rejects instructions with >1 inline sync wait)."""

    def _drain_and_barrier(self, tick_clock, wait_clock):
        nc = self.nc
        probe = nc.sync.nop(nofuse=True)
        wait_clock.add_sem_waits(probe.ins,
                                 ScopedClock({None: tick_clock.global_clock}))
        waits = list(probe.ins.sync_info.on_wait)
        probe.ins.sync_info = bass_rust.SyncInfo(on_wait=[], on_update=[])
        name2sem = {s.name: s for s in self.sems.allocated().values()}
        for w in waits:
            nc.sync.wait_ge(name2sem[w.ant_name], w.wait_value)
        nc.sync.drain()
        nc.all_engine_barrier()
        popped = nc._tile_sem_poison_stack.pop()
        assert popped is self._sem_poison
        nc.clear_and_free_semaphores(list(self.sems.allocated().values()))
        nc.all_engine_barrier()


def _split_bir_waits(bir_json: bytes) -> bytes:
    """Hoist all-but-one sync wait of any instruction onto standalone
    EventSemaphore ops placed just before it on the same engine queue."""
    d = json.loads(bir_json)
    changed = False
    for func in d.get("functions", []):
        for blk in func.get("blocks", []):
            out = []
            for inst in blk["instructions"]:
                si = inst.get("sync_info")
                waits = si.get("on_wait", []) if si else []
                if len(waits) > 1:
                    for k, w in enumerate(waits[:-1]):
                        out.append({
                            "name": f"{inst['name']}-hw{k}",
                            "opcode": "EventSemaphore",
                            "engine": inst["engine"],
                            "ins": [], "outs": [],
                            "sync_info": {"on_update": [], "on_wait": [w]},
                        })
                    si["on_wait"] = waits[-1:]
                    changed = True
                out.append(inst)
            blk["instructions"] = out
    if not changed:
        return bir_json
    return json.dumps(d).encode()


_hook_installed = False


def _install_wait_splitter():
    global _hook_installed
    if _hook_installed:
        return
    import concourse.bass2jax as bass2jax
    orig = bass2jax.compile_bir_kernel

    def patched(bir_json, tmpdir, neff_name="file.neff"):
        return orig(_split_bir_waits(bir_json), tmpdir, neff_name=neff_name)

    bass2jax.compile_bir_kernel = patched
    _hook_installed = True


def _wrap16(flat):
    """Pack a flat idx list into the 16-partition wrap layout [128, n/16]."""
    w = np.asarray(flat, np.int16).reshape(-1, 16).T     # [16, n/16]
    return np.tile(w, (8, 1))                            # [128, n/16]


# ---------------------------------------------------------------- host prep
def _host_prep(hidden, rela_embed, q_rel, edges):
    """Sort/shard/pad on the host. Returns per-core metadata arrays and the
    static per-tile section sizes (shared by all cores -> one SPMD program).

    C_list[t] = (cE, cO): chunks of edges whose sub row sits in the low /
    high half of the hidden table. Within a tile, slots are laid out
    [E-section | O-section], each padded to a chunk multiple.
    """
    r_idx = edges[:, 0].astype(np.int64)
    rel = edges[:, 2].astype(np.int64)
    sub = edges[:, 4].astype(np.int64)
    obj = edges[:, 5].astype(np.int64)
    q_rel = np.asarray(q_rel, np.int64)

    order = np.argsort(obj, kind="stable")
    obj_s = obj[order]
    sub_s = sub[order]
    rel_s = rel[order]
    qc_s = q_rel[r_idx[order]]        # composed: h_qr = rela[q_rel[r_idx]]

    # node_group: last write in ORIGINAL edge order (matches reference)
    node_group = np.zeros(N_PAD, np.int64)
    node_group[obj] = r_idx

    counts = np.bincount(obj_s, minlength=N_PAD)
    starts = np.zeros(N_PAD + 1, np.int64)
    np.cumsum(counts, out=starts[1:])

    # per-(core, tile) edge lists split by sub half
    per_ct = {}
    nE = np.zeros((NCORES, T_TILES), np.int64)
    nO = np.zeros((NCORES, T_TILES), np.int64)
    for core in range(NCORES):
        for t in range(T_TILES):
            g = core * T_TILES + t
            lo, hi = starts[g * P], starts[(g + 1) * P]
            sl = slice(lo, hi)
            isE = sub_s[sl] < NSPLIT
            per_ct[(core, t)] = (sub_s[sl], rel_s[sl], qc_s[sl],
                                 obj_s[sl] - g * P, isE)
            nE[core, t] = int(isE.sum())
            nO[core, t] = int((~isE).sum())

    C_list = []
    for t in range(T_TILES):
        cE = int(np.ceil(nE[:, t].max() / P))
        cO = int(np.ceil(nO[:, t].max() / P))
        if cE + cO == 0:
            cE = 1
        C_list.append((cE, cO))

    C2 = [cE + cO for cE, cO in C_list]
    col_off = np.zeros(T_TILES + 1, np.int64)
    np.cumsum(C2, out=col_off[1:])
    CT = int(col_off[-1])

    # idx-array column offsets (16 idx per col)
    hs_w = [C2[t] * 8 for t in range(T_TILES)]           # E then O sections
    hs_off = np.zeros(T_TILES + 1, np.int64)
    np.cumsum(hs_w, out=hs_off[1:])
    HSW = int(hs_off[-1])
    rq_w = [2 * C2[t] * 8 for t in range(T_TILES)]       # [relE qcE relO qcO]
    rq_off = np.zeros(T_TILES + 1, np.int64)
    np.cumsum(rq_w, out=rq_off[1:])
    RQW = int(rq_off[-1])

    obj32 = np.full((NCORES, P, CT), -1.0, np.float32)
    hsidx = np.zeros((NCORES, P, HSW), np.int16)
    rqidx = np.zeros((NCORES, P, RQW), np.int16)
    hnqidx = np.zeros((NCORES, P, NODES_PER_CORE // 16), np.int16)

    for core in range(NCORES):
        for t in range(T_TILES):
            cE, cO = C_list[t]
            sub_t, rel_t, qc_t, objl_t, isE = per_ct[(core, t)]
            subs = {}
            for sec, (base_c, sc, mask, boff) in enumerate(
                    [(0, cE, isE, 0), (cE, cO, ~isE, NSPLIT)]):
                if sc == 0:
                    continue
                n = int(mask.sum())
                sw = sc * P
                shs = np.zeros(sw, np.int16)
                srel = np.zeros(sw, np.int16)
                sqc = np.zeros(sw, np.int16)
                sobj = np.full(sw, -1.0, np.float32)
                shs[:n] = (sub_t[mask] - boff).astype(np.int16)
                srel[:n] = rel_t[mask].astype(np.int16)
                sqc[:n] = qc_t[mask].astype(np.int16)
                sobj[:n] = objl_t[mask].astype(np.float32)
                subs[sec] = (shs, srel, sqc, sobj, base_c)

            for sec, (shs, srel, sqc, sobj, base_c) in subs.items():
                sc = len(shs) // P
                c0 = int(col_off[t]) + base_c
                obj32[core, :, c0:c0 + sc] = sobj.reshape(sc, P).T
                h0 = int(hs_off[t]) + base_c * 8
                hsidx[core, :, h0:h0 + sc * 8] = _wrap16(shs)
                r0 = int(rq_off[t]) + 2 * base_c * 8
                rqidx[core, :, r0:r0 + 2 * sc * 8] = _wrap16(
                    np.concatenate([srel, sqc]))

        # h_n_qr gather idx: desc i -> (p=i%128, t=i//128)
        ng = node_group.reshape(NCORES, T_TILES, P)[core]   # [t, p]
        flat = ng.reshape(-1).astype(np.int16)              # i = t*128+p
        hnqidx[core] = _wrap16(flat)

    return dict(
        C_list=C_list, col_off=col_off, CT=CT,
        obj32=obj32, hsidx=hsidx, rqidx=rqidx, hnqidx=hnqidx,
    )


# ------------------------------------------------------------ device program
def _build_program(C_list, col_off, CT):
    C2 = [cE + cO for cE, cO in C_list]
    hs_w = [c * 8 for c in C2]
    hs_off = np.zeros(T_TILES + 1, np.int64)
    np.cumsum(hs_w, out=hs_off[1:])
    HSW = int(hs_off[-1])
    rq_w = [2 * c * 8 for c in C2]
    rq_off = np.zeros(T_TILES + 1, np.int64)
    np.cumsum(rq_w, out=rq_off[1:])
    RQW = int(rq_off[-1])
    Cmax = max(C2)

    nc = bass.Bass(dynamic_dma_scratch_size=32768, num_swdge_queues=1)
    dp = nc.declare_dram_parameter

    hidE = dp("hidE", [NSPLIT, D], f16, isOutput=False)
    hidO = dp("hidO", [N_NODE - NSPLIT, D], f16, isOutput=False)
    relap = dp("relap", [NREP, D], f16, isOutput=False)
    hq16 = dp("hq16", [NQ, D], f16, isOutput=False)

    wz_t = dp("wz_t", [D, D], f16, isOutput=False)
    wz_b = dp("wz_b", [D, D], f16, isOutput=False)
    uz = dp("uz", [D, D], f16, isOutput=False)
    wr_t = dp("wr_t", [D, D], f16, isOutput=False)
    wr_b = dp("wr_b", [D, D], f16, isOutput=False)
    ur = dp("ur", [D, D], f16, isOutput=False)
    wh_t = dp("wh_t", [D, D], f16, isOutput=False)
    wh_b = dp("wh_b", [D, D], f16, isOutput=False)
    uh = dp("uh", [D, D], f16, isOutput=False)
    ws = dp("ws", [D, A], f16, isOutput=False)
    wqr = dp("wqr", [D, A], f16, isOutput=False)
    walpha = dp("walpha", [A, 1], f16, isOutput=False)
    wh_out = dp("wh_out", [D, D], f16, isOutput=False)
    bz = dp("bz", [D, 1], f32, isOutput=False)
    br = dp("br", [D, 1], f32, isOutput=False)
    bh = dp("bh", [D, 1], f32, isOutput=False)
    bqr = dp("bqr", [A, 1], f32, isOutput=False)
    balpha = dp("balpha", [P, 1], f32, isOutput=False)
    iota16_d = dp("iota16", [P, P], f16, isOutput=False)
    ident16_d = dp("ident16", [P, P], f16, isOutput=False)

    obj32_d = dp("obj32", [P, CT], f32, isOutput=False)
    hsidx_d = dp("hsidx", [P, HSW], i16, isOutput=False)
    rqidx_d = dp("rqidx", [P, RQW], i16, isOutput=False)
    hnqidx_d = dp("hnqidx", [P, NODES_PER_CORE // 16], i16, isOutput=False)

    out_ht = dp("out_ht", [P, T_TILES * P], f32, isOutput=True)
    out_hnqr = dp("out_hnqr", [P, T_TILES * P], f16, isOutput=True)

    RING = 2048    # swdge ring capacity (scratch / 16)

    # pre-allocate num_idxs registers (to_reg needs the pool outside the TC)
    nidx_vals = set()
    for t in range(T_TILES):
        cE, cO = C_list[t]
        for sc in (cE, cO):
            if sc == 0:
                continue
            nidx_vals.add(sc * P)
            if 2 * sc * P <= RING:
                nidx_vals.add(2 * sc * P)
    # h_n_qr gather batches
    hnq_bat = []
    t0 = 0
    while t0 < T_TILES:
        tb = min(T_TILES - t0, RING // P)
        tb = min(tb, 13)
        hnq_bat.append((t0, tb))
        nidx_vals.add(tb * P)
        t0 += tb
    nidx_regs = {v: nc.gpsimd.to_reg(v) for v in sorted(nidx_vals)}

    from contextlib import ExitStack
    with _TC(nc) as tc, ExitStack() as ctx:
        const = ctx.enter_context(tc.tile_pool(name="const", bufs=1))
        meta = ctx.enter_context(tc.tile_pool(name="meta", bufs=1))
        gat = ctx.enter_context(tc.tile_pool(name="gat", bufs=3))
        rqp = ctx.enter_context(tc.tile_pool(name="rqp", bufs=3))
        mac = ctx.enter_context(tc.tile_pool(name="mac", bufs=6))
        pwp = ctx.enter_context(tc.tile_pool(name="pwp", bufs=6))
        fin = ctx.enter_context(tc.tile_pool(name="fin", bufs=2))
        p_g = ctx.enter_context(tc.tile_pool(name="p_g", bufs=NSTREAM,
                                             space="PSUM"))
        p_ag = ctx.enter_context(tc.tile_pool(name="p_ag", bufs=2,
                                              space="PSUM"))

        def load(pool, dram_t, shape, dt, tag):
            t = pool.tile(shape, dt, tag=tag)
            nc.sync.dma_start(t[:], dram_t[:])
            return t

        wz_t_s = load(const, wz_t, [D, D], f16, "wz_t")
        wz_b_s = load(const, wz_b, [D, D], f16, "wz_b")
        uz_s = load(const, uz, [D, D], f16, "uz")
        wr_t_s = load(const, wr_t, [D, D], f16, "wr_t")
        wr_b_s = load(const, wr_b, [D, D], f16, "wr_b")
        ur_s = load(const, ur, [D, D], f16, "ur")
        wh_t_s = load(const, wh_t, [D, D], f16, "wh_t")
        wh_b_s = load(const, wh_b, [D, D], f16, "wh_b")
        uh_s = load(const, uh, [D, D], f16, "uh")
        ws_s = load(const, ws, [D, A], f16, "ws")
        wqr_s = load(const, wqr, [D, A], f16, "wqr")
        walpha_s = load(const, walpha, [A, 1], f16, "walpha")
        whout_s = load(const, wh_out, [D, D], f16, "whout")
        bz_s = load(const, bz, [D, 1], f32, "bz")
        br_s = load(const, br, [D, 1], f32, "br")
        bh_s = load(const, bh, [D, 1], f32, "bh")
        bqr_s = load(const, bqr, [A, 1], f32, "bqr")
        balpha_s = load(const, balpha, [P, 1], f32, "balpha")
        iota16_s = load(const, iota16_d, [P, P], f16, "iota16")
        ident16_s = load(const, ident16_d, [P, P], f16, "ident16")

        obj32_s = load(meta, obj32_d, [P, CT], f32, "obj32")
        hsidx_s = load(meta, hsidx_d, [P, HSW], i16, "hsidx")
        rqidx_s = load(meta, rqidx_d, [P, RQW], i16, "rqidx")
        hnqidx_s = load(meta, hnqidx_d, [P, NODES_PER_CORE // 16], i16,
                        "hnqidx")

        nc.gpsimd.load_library(library_config.mlp)

        mm = nc.tensor.matmul
        act = nc.scalar.activation

        def gather_T(out_sl, table, idx_sl, n):
            """dma_gather transpose=True: rows -> feature-major columns."""
            nc.gpsimd.dma_gather(
                out_ap=out_sl.rearrange("p (k e) -> p k e", k=1),
                in_ap=table[:],
                idxs_ap=idx_sl,
                num_idxs=n, num_idxs_reg=nidx_regs[n],
                elem_size=D, transpose=True, single_packet=False)

        # ---- per-tile fetch: hs from the two hidden halves + rela rows
        def emit_fetch(t):
            cE, cO = C_list[t]
            C2t = cE + cO
            hsT = gat.tile([P, Cmax * P], f16, tag="hsT")
            rqT = rqp.tile([P, 2 * Cmax * P], f16, tag="rqT")
            h0 = int(hs_off[t])
            r0 = int(rq_off[t])
            if "fetch" in DISABLE:
                nc.vector.memset(hsT[:], 0.25)
                nc.vector.memset(rqT[:], 0.25)
                return hsT, rqT
            for base_c, sc, table in ((0, cE, hidE), (cE, cO, hidO)):
                if sc == 0:
                    continue
                sw = sc * P
                gather_T(hsT[:, base_c * P:base_c * P + sw], table,
                         hsidx_s[:, h0 + base_c * 8:h0 + (base_c + sc) * 8],
                         sw)
                rsl = rqidx_s[:, r0 + 2 * base_c * 8:
                              r0 + 2 * (base_c + sc) * 8]
                if 2 * sw <= RING:
                    gather_T(rqT[:, 2 * base_c * P:2 * base_c * P + 2 * sw],
                             relap, rsl, 2 * sw)
                else:
                    gather_T(rqT[:, 2 * base_c * P:2 * base_c * P + sw],
                             relap, rsl[:, :sc * 8], sw)
                    gather_T(rqT[:, 2 * base_c * P + sw:
                                  2 * base_c * P + 2 * sw],
                             relap, rsl[:, sc * 8:], sw)
            return hsT, rqT

        fetched = {t: emit_fetch(t) for t in range(min(PF_TILES, T_TILES))}

        # ---- h_n_qr output: batched hq gather -> DRAM store
        hnq_sb = const.tile([P, T_TILES * P], f16, tag="hnq")
        if "hnq" in DISABLE:
            nc.vector.memset(hnq_sb[:], 0.0)
        for (b0, tb) in ([] if "hnq" in DISABLE else hnq_bat):
            nc.gpsimd.dma_gather(
                out_ap=hnq_sb[:, b0 * P:(b0 + tb) * P].rearrange(
                    "p (t d) -> p t d", d=P),
                in_ap=hq16[:],
                idxs_ap=hnqidx_s[:, b0 * 8:(b0 + tb) * 8],
                num_idxs=tb * P, num_idxs_reg=nidx_regs[tb * P],
                elem_size=D, transpose=False, single_packet=False)
        nc.sync.dma_start(out_hnqr[:], hnq_sb[:])

        # ---- macro pipeline stages as a generator (one PSUM bank / stream)
        tile_state = {}

        def macro_gen(t, base_c, sc, m0, mc, first_alpha, last_of_tile):
            st = tile_state[t]
            hsT, rqT, agg = st["hsT"], st["rqT"], st["agg"]
            co = int(col_off[t])
            g0 = base_c + m0                 # global chunk within tile
            ew = mc * P
            hs_sl = hsT[:, (base_c + m0) * P:(base_c + m0) * P + ew]
            hr_sl = rqT[:, (2 * base_c + m0) * P:
                        (2 * base_c + m0) * P + ew]
            hq_sl = rqT[:, (2 * base_c + sc + m0) * P:
                        (2 * base_c + sc + m0) * P + ew]

            G = p_g.tile([P, 512], f32, tag="G")
            G16 = G[:].bitcast(f16)

            mm(G[:, :ew], lhsT=wz_t_s[:], rhs=hr_sl, start=True, stop=False)
            mm(G[:, :ew], lhsT=wz_b_s[:], rhs=hq_sl, start=False, stop=False)
            mm(G[:, :ew], lhsT=uz_s[:], rhs=hs_sl, start=False, stop=True)
            yield
            z_sb = mac.tile([P, MACRO * P], f16, tag="z")
            act(z_sb[:, :ew], G[:, :ew], AF.Sigmoid, bias=bz_s[:, :1])
            yield
            mm(G[:, :ew], lhsT=wr_t_s[:], rhs=hr_sl, start=True, stop=False)
            mm(G[:, :ew], lhsT=wr_b_s[:], rhs=hq_sl, start=False, stop=False)
            mm(G[:, :ew], lhsT=ur_s[:], rhs=hs_sl, start=False, stop=True)
            yield
            r_sb = mac.tile([P, MACRO * P], f16, tag="r")
            act(r_sb[:, :ew], G[:, :ew], AF.Sigmoid, bias=br_s[:, :1])
            yield
            rh = mac.tile([P, MACRO * P], f16, tag="rh")
            nc.vector.tensor_tensor(out=rh[:, :ew], in0=r_sb[:, :ew],
                                    in1=hs_sl, op=ALU.mult)
            yield
            mm(G[:, :ew], lhsT=wh_t_s[:], rhs=hr_sl, start=True, stop=False)
            mm(G[:, :ew], lhsT=wh_b_s[:], rhs=hq_sl, start=False, stop=False)
            mm(G[:, :ew], lhsT=uh_s[:], rhs=rh[:, :ew], start=False,
               stop=True)
            yield
            ht = mac.tile([P, MACRO * P], f16, tag="ht")
            act(ht[:, :ew], G[:, :ew], AF.Tanh, bias=bh_s[:, :1])
            yield
            dd = mac.tile([P, MACRO * P], f16, tag="dd")
            nc.vector.tensor_tensor(out=dd[:, :ew], in0=ht[:, :ew],
                                    in1=hs_sl, op=ALU.subtract)
            zd = mac.tile([P, MACRO * P], f16, tag="zd")
            nc.vector.tensor_tensor(out=zd[:, :ew], in0=z_sb[:, :ew],
                                    in1=dd[:, :ew], op=ALU.mult)
            msgT = mac.tile([P, MACRO * P], f16, tag="msgT")
            nc.vector.tensor_tensor(out=msgT[:, :ew], in0=zd[:, :ew],
                                    in1=hs_sl, op=ALU.add)
            yield
            mm(G[:, :ew], lhsT=ws_s[:], rhs=msgT[:, :ew], start=True,
               stop=False)
            mm(G[:, :ew], lhsT=wqr_s[:], rhs=hq_sl, start=False, stop=True)
            yield
            relu_sb = mac.tile([P, MACRO * P], f16, tag="relu")
            nc.vector.tensor_scalar(
                out=relu_sb[:, :ew], in0=G[:, :ew],
                scalar1=bqr_s[:, :1], scalar2=0.0,
                op0=ALU.add, op1=ALU.max)
            yield
            for c in range(mc):
                col = 140 + g0 + c
                mm(agg[:, col:col + 1],
                   lhsT=relu_sb[:, c * P:(c + 1) * P], rhs=walpha_s[:],
                   start=(first_alpha and c == 0), stop=True,
                   skip_group_check=True)
            yield
            expc = mac.tile([P, MACRO], f32, tag="expc")
            act(expc[:, :mc], agg[:, 140 + g0:140 + g0 + mc], AF.Exp,
                bias=balpha_s[:, :1])
            yield
            pw = pwp.tile([P, MACRO * P], f16, tag="pw")
            for c in range(mc):
                nc.vector.tensor_scalar(
                    out=pw[:, c * P:(c + 1) * P], in0=iota16_s[:],
                    scalar1=obj32_s[:, co + g0 + c:co + g0 + c + 1],
                    scalar2=expc[:, c:c + 1],
                    op0=ALU.is_equal, op1=ALU.mult)
            if "msgE_T" not in DISABLE:
                for c in range(mc):
                    mm(G16[:, c * P:(c + 1) * P],
                       lhsT=msgT[:, c * P:(c + 1) * P],
                       rhs=ident16_s[:], is_transpose=True,
                       start=(c == 0), stop=(c == mc - 1))
            yield
            msgE = pwp.tile([P, MACRO * 129], f16, tag="msgE")
            mview = msgE[:].rearrange("p (c x) -> p c x", x=129)
            nc.vector.memset(mview[:, :mc, 128:129], 1.0)
            if "msgE_T" in DISABLE:
                nc.vector.memset(mview[:, :mc, 0:128], 0.5)
            else:
                nc.vector.tensor_copy(
                    mview[:, :mc, 0:128],
                    G16[:, :ew].rearrange("p (c x) -> p c x", x=P))
            yield
            for c in range(mc):
                mm(agg[:, 0:129], lhsT=pw[:, c * P:(c + 1) * P],
                   rhs=mview[:, c, 0:129],
                   start=False, stop=(last_of_tile and c == mc - 1),
                   skip_group_check=True)
            if not last_of_tile:
                return
            # ---- finalize (only the tile's LAST macro reaches here, after
            # every other macro of the tile has emitted its agg matmuls)
            yield
            recip = fin.tile([P, 1], f32, tag="recip")
            nc.vector.reciprocal(recip[:], agg[:, 128:129])
            magg = fin.tile([P, P], f16, tag="magg")
            nc.vector.tensor_scalar(out=magg[:], in0=agg[:, 0:128],
                                    scalar1=recip[:, :1], scalar2=None,
                                    op0=ALU.mult)
            yield
            maggT = fin.tile([P, P], f16, tag="maggT")
            nc.sync.dma_start_transpose(out=maggT[:], in_=magg[:])
            yield
            mm(agg[:, 160:288], lhsT=whout_s[:], rhs=maggT[:],
               start=False, stop=True, skip_group_check=True)
            yield
            hnew = fin.tile([P, P], f32, tag="hnew")
            act(hnew[:], agg[:, 160:288], AF.Relu)
            yield
            nc.sync.dma_start(out_ht[:, t * P:(t + 1) * P], hnew[:])

        # ---- job list: per tile, macros split within each slot section
        jobs = []
        for t in range(T_TILES):
            cE, cO = C_list[t]
            C2t = cE + cO
            macros = []
            for base_c, sc in ((0, cE), (cE, cO)):
                m0 = 0
                while m0 < sc:
                    macros.append((base_c, sc, m0, min(MACRO, sc - m0)))
                    m0 += MACRO
            for k, (base_c, sc, m0, mc) in enumerate(macros):
                jobs.append(("m", t, base_c, sc, m0, mc, k == 0,
                             k == len(macros) - 1))

        # ---- stream scheduler: round-robin one stage per sweep, with
        # admission staggered so streams don't hit the same engine's
        # stage in the same sweep (STAGGER sweeps of initial delay)
        from collections import deque
        STAGGER = 3
        pending = deque(jobs)
        active = []          # [gen, delay]
        stag = 0
        while pending or active:
            while len(active) < NSTREAM and pending:
                job = pending.popleft()
                _, t, base_c, sc, m0, mc, first, last = job
                if t not in tile_state:
                    tile_state[t] = {"agg": None}
                    hsT, rqT = fetched.pop(t)
                    tile_state[t].update(hsT=hsT, rqT=rqT)
                    if t + PF_TILES < T_TILES:
                        fetched[t + PF_TILES] = emit_fetch(t + PF_TILES)
                if first:
                    tile_state[t]["agg"] = p_ag.tile(
                        [P, 512], f32, tag="agg", name="agg")
                g = macro_gen(t, base_c, sc, m0, mc, first, last)
                active.append([g, stag])
                stag += STAGGER
            stag = max(0, stag - 1)
            for ent in list(active):
                if ent[1] > 0:
                    ent[1] -= 1
                    continue
                try:
                    next(ent[0])
                except StopIteration:
                    active.remove(ent)

    return nc


# ----------------------------------------------------------------- kernel()
def kernel(hidden, rela_embed, Wz, Uz, bz, Wr_g, Ur, br, Whh, Uh, bh,
           Ws_attn, Wqr_attn, b_qr, w_alpha, b_alpha, W_h,
           q_rel, edges, n_node):
    _install_wait_splitter()

    hidden = np.asarray(hidden, np.float32)
    rela_embed = np.asarray(rela_embed, np.float32)
    edges = np.asarray(edges)
    q_rel = np.asarray(q_rel)

    meta = _host_prep(hidden, rela_embed, q_rel, edges)
    C_list, col_off, CT = meta["C_list"], meta["col_off"], meta["CT"]

    hq = rela_embed[np.asarray(q_rel, np.int64)]          # [NQ, D]
    relap = np.zeros((NREP, D), np.float32)
    relap[:NRE] = rela_embed

    nc = _build_program(C_list, col_off, CT)
    # lower InstISA subclasses (the gpsimd library-load pseudo op) to real
    # MODIFY_POOL_CONFIG encodings so walrus can compile them
    mybir.codegen_inst_isa_subclasses(nc)

    hid16 = hidden.astype(np.float16)
    common = {
        "hidE": hid16[:NSPLIT],
        "hidO": hid16[NSPLIT:],
        "relap": relap.astype(np.float16),
        "hq16": hq.astype(np.float16),
        "wz_t": np.asarray(Wz[:D], np.float16),
        "wz_b": np.asarray(Wz[D:], np.float16),
        "uz": np.asarray(Uz, np.float16),
        "wr_t": np.asarray(Wr_g[:D], np.float16),
        "wr_b": np.asarray(Wr_g[D:], np.float16),
        "ur": np.asarray(Ur, np.float16),
        "wh_t": np.asarray(Whh[:D], np.float16),
        "wh_b": np.asarray(Whh[D:], np.float16),
        "uh": np.asarray(Uh, np.float16),
        "ws": np.asarray(Ws_attn, np.float16),
        "wqr": np.asarray(Wqr_attn, np.float16),
        "walpha": np.asarray(w_alpha, np.float16).reshape(A, 1),
        "wh_out": np.asarray(W_h, np.float16),
        "bz": np.asarray(bz, np.float32).reshape(D, 1),
        "br": np.asarray(br, np.float32).reshape(D, 1),
        "bh": np.asarray(bh, np.float32).reshape(D, 1),
        "bqr": np.asarray(b_qr, np.float32).reshape(A, 1),
        "balpha": np.full((P, 1), float(np.asarray(b_alpha).reshape(-1)[0]),
                          np.float32),
        "iota16": np.broadcast_to(np.arange(P, dtype=np.float16),
                                  (P, P)).copy(),
        "ident16": np.eye(P, dtype=np.float16),
    }
    in_maps = []
    for core in range(NCORES):
        m = dict(common)
        m["obj32"] = meta["obj32"][core]
        m["hsidx"] = meta["hsidx"][core]
        m["rqidx"] = meta["rqidx"][core]
        m["hnqidx"] = meta["hnqidx"][core]
        in_maps.append(m)

    res = run_bass_kernel_spmd(nc, in_maps, list(range(NCORES))).results

    hidden_new = np.empty((N_PAD, D), np.float32)
    h_n_qr = np.empty((N_PAD, D), np.float32)
    for core in range(NCORES):
        lo = core * NODES_PER_CORE
        hi = lo + NODES_PER_CORE
        hidden_new[lo:hi] = res[core]["out_ht"].T
        h_n_qr[lo:hi] = (res[core]["out_hnqr"].astype(np.float32)
                         .reshape(P, T_TILES, P).transpose(1, 0, 2)
                         .reshape(NODES_PER_CORE, D))

    return hidden_new[:N_NODE], h_n_qr[:N_NODE]


# revision 27
# speedup vs baseline: 3.1121x; 1.5146x over previous
"""Trainium2 Bass kernel for nn_RRE_GNN_raw (GNN message passing), v3.

Strategy: sort edges by destination node (obj) on the host, shard NODES
across the 8 cores (each core owns 49 node-tiles of 128 nodes and all
edges pointing into them -> no collectives). All per-edge row gathers
use int16 DMAGather with transpose=True, which lands rows directly
FEATURE-major in SBUF:
  - hidden[sub] is gathered from two <=32768-row halves of the hidden
    table (int16 index limit); each tile's edge slots are grouped by
    half so every gather call covers one contiguous slot range,
  - h_r = rela[rel] and h_qr = rela[q_rel[r_idx]] come from the 401-row
    rela table in one call per slot section.
Compute runs in macros of up to 4x128 edges: 9 gate matmuls + 2
attention matmuls at macro width, activations/DVE ops at macro width,
message transposed back edge-major with f16 PE transposes, and the
softmax-weighted segment sums accumulate in PSUM per node-tile via
scaled one-hot matmuls (ones column folded into the 129-wide matmul).

Scheduling: ~6 macro "streams" advance round-robin one stage per sweep,
each stream owning ONE rotating PSUM bank (zp->rp->hp->apre->msgE
phases reuse it), so every engine's in-order queue always holds ready
work from other streams and head-of-line blocking is minimized.

The h_n_qr output is produced by batched DMAGather at kernel start.
"""
import sys

sys.path.insert(0, '/opt/trn_rl_repo')

import json
import numpy as np

import concourse.bass as bass
import concourse.tile as tile
from concourse import library_config
from concourse import mybir
from concourse.bass_utils import run_bass_kernel_spmd
from concourse.vector_clock import ScopedClock
import bass_rust

# ---------------------------------------------------------------- constants
P = 128            # partitions / tile edge
D = 128            # feature dim
A = 128            # attention dim
N_NODE = 50000
NSPLIT = 32768     # int16 index limit for dma_gather tables
NQ = 1024
NRE = 401
NREP = 512         # rela table padded
NCORES = 8
T_TILES = 49       # node tiles per core
NODES_PER_CORE = T_TILES * P          # 6272
N_PAD = NCORES * NODES_PER_CORE       # 50176
MACRO = 4          # chunks per macro (512 edges)
NSTREAM = 6        # concurrent macro streams (PSUM G banks)
AGG_BUFS = 2       # PSUM agg banks (NSTREAM + AGG_BUFS <= 8)
RELU_SPLIT = 3     # every RELU_SPLIT-th macro does relu on Act instead of DVE
MAC_BUFS = 6       # SBUF rotation depth for per-macro tiles
STAGGER = 0        # sweeps of admission stagger between streams
PF_TILES = 3       # gather prefetch depth in tiles

f16 = mybir.dt.float16
f32 = mybir.dt.float32
i32 = mybir.dt.int32
i16 = mybir.dt.int16

DISABLE = set()          # debug bisection knobs
AF = mybir.ActivationFunctionType
ALU = mybir.AluOpType


# ------------------------------------------------- harness compatibility fixes
class _TC(tile.TileContext):
    """TileContext whose kernel-tail drain emits one wait per instruction
    (the walrus build here rejects instructions with >1 inline sync wait)."""

    def _drain_and_barrier(self, tick_clock, wait_clock):
        nc = self.nc
        probe = nc.sync.nop(nofuse=True)
        wait_clock.add_sem_waits(probe.ins,
                                 ScopedClock({None: tick_clock.global_clock}))
        waits = list(probe.ins.sync_info.on_wait)
        probe.ins.sync_info = bass_rust.SyncInfo(on_wait=[], on_update=[])
        name2sem = {s.name: s for s in self.sems.allocated().values()}
        for w in waits:
            nc.sync.wait_ge(name2sem[w.ant_name], w.wait_value)
        nc.sync.drain()
        nc.all_engine_barrier()
        popped = nc._tile_sem_poison_stack.pop()
        assert popped is self._sem_poison
        nc.clear_and_free_semaphores(list(self.sems.allocated().values()))
        nc.all_engine_barrier()


def _split_bir_waits(bir_json: bytes) -> bytes:
    """Hoist all-but-one sync wait of any instruction onto standalone
    EventSemaphore ops placed just before it on the same engine queue."""
    d = json.loads(bir_json)
    changed = False
    for func in d.get("functions", []):
        for blk in func.get("blocks", []):
            out = []
            for inst in blk["instructions"]:
                si = inst.get("sync_info")
                waits = si.get("on_wait", []) if si else []
                if len(waits) > 1:
                    for k, w in enumerate(waits[:-1]):
                        out.append({
                            "name": f"{inst['name']}-hw{k}",
                            "opcode": "EventSemaphore",
                            "engine": inst["engine"],
                            "ins": [], "outs": [],
                            "sync_info": {"on_update": [], "on_wait": [w]},
                        })
                    si["on_wait"] = waits[-1:]
                    changed = True
                out.append(inst)
            blk["instructions"] = out
    if not changed:
        return bir_json
    return json.dumps(d).encode()


_hook_installed = False


def _install_wait_splitter():
    global _hook_installed
    if _hook_installed:
        return
    import concourse.bass2jax as bass2jax
    orig = bass2jax.compile_bir_kernel

    def patched(bir_json, tmpdir, neff_name="file.neff"):
        return orig(_split_bir_waits(bir_json), tmpdir, neff_name=neff_name)

    bass2jax.compile_bir_kernel = patched
    _hook_installed = True


def _wrap16(flat):
    """Pack a flat idx list into the 16-partition wrap layout [128, n/16]."""
    w = np.asarray(flat, np.int16).reshape(-1, 16).T     # [16, n/16]
    return np.tile(w, (8, 1))                            # [128, n/16]


# ---------------------------------------------------------------- host prep
def _host_prep(hidden, rela_embed, q_rel, edges):
    """Sort/shard/pad on the host. Returns per-core metadata arrays and the
    static per-tile section sizes (shared by all cores -> one SPMD program).

    C_list[t] = (cE, cO): chunks of edges whose sub row sits in the low /
    high half of the hidden table. Within a tile, slots are laid out
    [E-section | O-section], each padded to a chunk multiple.
    """
    r_idx = edges[:, 0].astype(np.int64)
    rel = edges[:, 2].astype(np.int64)
    sub = edges[:, 4].astype(np.int64)
    obj = edges[:, 5].astype(np.int64)
    q_rel = np.asarray(q_rel, np.int64)

    order = np.argsort(obj, kind="stable")
    obj_s = obj[order]
    sub_s = sub[order]
    rel_s = rel[order]
    qc_s = q_rel[r_idx[order]]        # composed: h_qr = rela[q_rel[r_idx]]

    # node_group: last write in ORIGINAL edge order (matches reference)
    node_group = np.zeros(N_PAD, np.int64)
    node_group[obj] = r_idx

    counts = np.bincount(obj_s, minlength=N_PAD)
    starts = np.zeros(N_PAD + 1, np.int64)
    np.cumsum(counts, out=starts[1:])

    # per-(core, tile) edge lists split by sub half
    per_ct = {}
    nE = np.zeros((NCORES, T_TILES), np.int64)
    nO = np.zeros((NCORES, T_TILES), np.int64)
    for core in range(NCORES):
        for t in range(T_TILES):
            g = core * T_TILES + t
            lo, hi = starts[g * P], starts[(g + 1) * P]
            sl = slice(lo, hi)
            isE = sub_s[sl] < NSPLIT
            per_ct[(core, t)] = (sub_s[sl], rel_s[sl], qc_s[sl],
                                 obj_s[sl] - g * P, isE)
            nE[core, t] = int(isE.sum())
            nO[core, t] = int((~isE).sum())

    C_list = []
    for t in range(T_TILES):
        cE = int(np.ceil(nE[:, t].max() / P))
        cO = int(np.ceil(nO[:, t].max() / P))
        if cE + cO == 0:
            cE = 1
        C_list.append((cE, cO))

    C2 = [cE + cO for cE, cO in C_list]
    col_off = np.zeros(T_TILES + 1, np.int64)
    np.cumsum(C2, out=col_off[1:])
    CT = int(col_off[-1])

    # idx-array column offsets (16 idx per col)
    hs_w = [C2[t] * 8 for t in range(T_TILES)]           # E then O sections
    hs_off = np.zeros(T_TILES + 1, np.int64)
    np.cumsum(hs_w, out=hs_off[1:])
    HSW = int(hs_off[-1])
    rq_w = [2 * C2[t] * 8 for t in range(T_TILES)]       # [relE qcE relO qcO]
    rq_off = np.zeros(T_TILES + 1, np.int64)
    np.cumsum(rq_w, out=rq_off[1:])
    RQW = int(rq_off[-1])

    obj32 = np.full((NCORES, P, CT), -1.0, np.float32)
    hsidx = np.zeros((NCORES, P, HSW), np.int16)
    rqidx = np.zeros((NCORES, P, RQW), np.int16)
    hnqidx = np.zeros((NCORES, P, NODES_PER_CORE // 16), np.int16)

    for core in range(NCORES):
        for t in range(T_TILES):
            cE, cO = C_list[t]
            sub_t, rel_t, qc_t, objl_t, isE = per_ct[(core, t)]
            subs = {}
            for sec, (base_c, sc, mask, boff) in enumerate(
                    [(0, cE, isE, 0), (cE, cO, ~isE, NSPLIT)]):
                if sc == 0:
                    continue
                n = int(mask.sum())
                sw = sc * P
                shs = np.zeros(sw, np.int16)
                srel = np.zeros(sw, np.int16)
                sqc = np.zeros(sw, np.int16)
                sobj = np.full(sw, -1.0, np.float32)
                shs[:n] = (sub_t[mask] - boff).astype(np.int16)
                srel[:n] = rel_t[mask].astype(np.int16)
                sqc[:n] = qc_t[mask].astype(np.int16)
                sobj[:n] = objl_t[mask].astype(np.float32)
                subs[sec] = (shs, srel, sqc, sobj, base_c)

            for sec, (shs, srel, sqc, sobj, base_c) in subs.items():
                sc = len(shs) // P
                c0 = int(col_off[t]) + base_c
                obj32[core, :, c0:c0 + sc] = sobj.reshape(sc, P).T
                h0 = int(hs_off[t]) + base_c * 8
                hsidx[core, :, h0:h0 + sc * 8] = _wrap16(shs)
                r0 = int(rq_off[t]) + 2 * base_c * 8
                rqidx[core, :, r0:r0 + 2 * sc * 8] = _wrap16(
                    np.concatenate([srel, sqc]))

        # h_n_qr gather idx: desc i -> (p=i%128, t=i//128)
        ng = node_group.reshape(NCORES, T_TILES, P)[core]   # [t, p]
        flat = ng.reshape(-1).astype(np.int16)              # i = t*128+p
        hnqidx[core] = _wrap16(flat)

    return dict(
        C_list=C_list, col_off=col_off, CT=CT,
        obj32=obj32, hsidx=hsidx, rqidx=rqidx, hnqidx=hnqidx,
    )


# ------------------------------------------------------------ device program
def _build_program(C_list, col_off, CT):
    C2 = [cE + cO for cE, cO in C_list]
    hs_w = [c * 8 for c in C2]
    hs_off = np.zeros(T_TILES + 1, np.int64)
    np.cumsum(hs_w, out=hs_off[1:])
    HSW = int(hs_off[-1])
    rq_w = [2 * c * 8 for c in C2]
    rq_off = np.zeros(T_TILES + 1, np.int64)
    np.cumsum(rq_w, out=rq_off[1:])
    RQW = int(rq_off[-1])
    Cmax = max(C2)

    nc = bass.Bass(dynamic_dma_scratch_size=32768, num_swdge_queues=1)
    dp = nc.declare_dram_parameter

    hidE = dp("hidE", [NSPLIT, D], f16, isOutput=False)
    hidO = dp("hidO", [N_NODE - NSPLIT, D], f16, isOutput=False)
    relap = dp("relap", [NREP, D], f16, isOutput=False)
    hq16 = dp("hq16", [NQ, D], f16, isOutput=False)

    wz_t = dp("wz_t", [D, D], f16, isOutput=False)
    wz_b = dp("wz_b", [D, D], f16, isOutput=False)
    uz = dp("uz", [D, D], f16, isOutput=False)
    wr_t = dp("wr_t", [D, D], f16, isOutput=False)
    wr_b = dp("wr_b", [D, D], f16, isOutput=False)
    ur = dp("ur", [D, D], f16, isOutput=False)
    wh_t = dp("wh_t", [D, D], f16, isOutput=False)
    wh_b = dp("wh_b", [D, D], f16, isOutput=False)
    uh = dp("uh", [D, D], f16, isOutput=False)
    ws = dp("ws", [D, A], f16, isOutput=False)
    wqr = dp("wqr", [D, A], f16, isOutput=False)
    walpha = dp("walpha", [A, 1], f16, isOutput=False)
    wh_out = dp("wh_out", [D, D], f16, isOutput=False)
    bz = dp("bz", [D, 1], f32, isOutput=False)
    br = dp("br", [D, 1], f32, isOutput=False)
    bh = dp("bh", [D, 1], f32, isOutput=False)
    bqr = dp("bqr", [A, 1], f32, isOutput=False)
    balpha = dp("balpha", [P, 1], f32, isOutput=False)
    iota16_d = dp("iota16", [P, P], f16, isOutput=False)
    ident16_d = dp("ident16", [P, P], f16, isOutput=False)

    obj32_d = dp("obj32", [P, CT], f32, isOutput=False)
    hsidx_d = dp("hsidx", [P, HSW], i16, isOutput=False)
    rqidx_d = dp("rqidx", [P, RQW], i16, isOutput=False)
    hnqidx_d = dp("hnqidx", [P, NODES_PER_CORE // 16], i16, isOutput=False)

    out_ht = dp("out_ht", [P, T_TILES * P], f32, isOutput=True)
    out_hnqr = dp("out_hnqr", [P, T_TILES * P], f16, isOutput=True)

    RING = 2048    # swdge ring capacity (scratch / 16)

    # pre-allocate num_idxs registers (to_reg needs the pool outside the TC)
    nidx_vals = set()
    for t in range(T_TILES):
        cE, cO = C_list[t]
        for sc in (cE, cO):
            if sc == 0:
                continue
            nidx_vals.add(sc * P)
            if 2 * sc * P <= RING:
                nidx_vals.add(2 * sc * P)
    # h_n_qr gather batches
    hnq_bat = []
    t0 = 0
    while t0 < T_TILES:
        tb = min(T_TILES - t0, RING // P)
        tb = min(tb, 13)
        hnq_bat.append((t0, tb))
        nidx_vals.add(tb * P)
        t0 += tb
    nidx_regs = {v: nc.gpsimd.to_reg(v) for v in sorted(nidx_vals)}

    from contextlib import ExitStack
    with _TC(nc) as tc, ExitStack() as ctx:
        const = ctx.enter_context(tc.tile_pool(name="const", bufs=1))
        meta = ctx.enter_context(tc.tile_pool(name="meta", bufs=1))
        gat = ctx.enter_context(tc.tile_pool(name="gat", bufs=PF_TILES))
        rqp = ctx.enter_context(tc.tile_pool(name="rqp", bufs=PF_TILES))
        mac = ctx.enter_context(tc.tile_pool(name="mac", bufs=MAC_BUFS))
        pwp = ctx.enter_context(tc.tile_pool(name="pwp", bufs=MAC_BUFS))
        fin = ctx.enter_context(tc.tile_pool(name="fin", bufs=2))
        p_g = ctx.enter_context(tc.tile_pool(name="p_g", bufs=NSTREAM,
                                             space="PSUM"))
        p_ag = ctx.enter_context(tc.tile_pool(name="p_ag", bufs=AGG_BUFS,
                                              space="PSUM"))

        def load(pool, dram_t, shape, dt, tag):
            t = pool.tile(shape, dt, tag=tag)
            nc.sync.dma_start(t[:], dram_t[:])
            return t

        wz_t_s = load(const, wz_t, [D, D], f16, "wz_t")
        wz_b_s = load(const, wz_b, [D, D], f16, "wz_b")
        uz_s = load(const, uz, [D, D], f16, "uz")
        wr_t_s = load(const, wr_t, [D, D], f16, "wr_t")
        wr_b_s = load(const, wr_b, [D, D], f16, "wr_b")
        ur_s = load(const, ur, [D, D], f16, "ur")
        wh_t_s = load(const, wh_t, [D, D], f16, "wh_t")
        wh_b_s = load(const, wh_b, [D, D], f16, "wh_b")
        uh_s = load(const, uh, [D, D], f16, "uh")
        ws_s = load(const, ws, [D, A], f16, "ws")
        wqr_s = load(const, wqr, [D, A], f16, "wqr")
        walpha_s = load(const, walpha, [A, 1], f16, "walpha")
        whout_s = load(const, wh_out, [D, D], f16, "whout")
        bz_s = load(const, bz, [D, 1], f32, "bz")
        br_s = load(const, br, [D, 1], f32, "br")
        bh_s = load(const, bh, [D, 1], f32, "bh")
        bqr_s = load(const, bqr, [A, 1], f32, "bqr")
        balpha_s = load(const, balpha, [P, 1], f32, "balpha")
        iota16_s = load(const, iota16_d, [P, P], f16, "iota16")
        ident16_s = load(const, ident16_d, [P, P], f16, "ident16")

        obj32_s = load(meta, obj32_d, [P, CT], f32, "obj32")
        hsidx_s = load(meta, hsidx_d, [P, HSW], i16, "hsidx")
        rqidx_s = load(meta, rqidx_d, [P, RQW], i16, "rqidx")
        hnqidx_s = load(meta, hnqidx_d, [P, NODES_PER_CORE // 16], i16,
                        "hnqidx")

        nc.gpsimd.load_library(library_config.mlp)

        mm = nc.tensor.matmul
        act = nc.scalar.activation

        def gather_T(out_sl, table, idx_sl, n):
            """dma_gather transpose=True: rows -> feature-major columns."""
            nc.gpsimd.dma_gather(
                out_ap=out_sl.rearrange("p (k e) -> p k e", k=1),
                in_ap=table[:],
                idxs_ap=idx_sl,
                num_idxs=n, num_idxs_reg=nidx_regs[n],
                elem_size=D, transpose=True, single_packet=False)

        # ---- per-tile fetch: hs from the two hidden halves + rela rows
        def emit_fetch(t):
            cE, cO = C_list[t]
            C2t = cE + cO
            hsT = gat.tile([P, Cmax * P], f16, tag="hsT")
            rqT = rqp.tile([P, 2 * Cmax * P], f16, tag="rqT")
            h0 = int(hs_off[t])
            r0 = int(rq_off[t])
            if "fetch" in DISABLE:
                nc.vector.memset(hsT[:], 0.25)
                nc.vector.memset(rqT[:], 0.25)
                return hsT, rqT
            for base_c, sc, table in ((0, cE, hidE), (cE, cO, hidO)):
                if sc == 0:
                    continue
                sw = sc * P
                gather_T(hsT[:, base_c * P:base_c * P + sw], table,
                         hsidx_s[:, h0 + base_c * 8:h0 + (base_c + sc) * 8],
                         sw)
                rsl = rqidx_s[:, r0 + 2 * base_c * 8:
                              r0 + 2 * (base_c + sc) * 8]
                if 2 * sw <= RING:
                    gather_T(rqT[:, 2 * base_c * P:2 * base_c * P + 2 * sw],
                             relap, rsl, 2 * sw)
                else:
                    gather_T(rqT[:, 2 * base_c * P:2 * base_c * P + sw],
                             relap, rsl[:, :sc * 8], sw)
                    gather_T(rqT[:, 2 * base_c * P + sw:
                                  2 * base_c * P + 2 * sw],
                             relap, rsl[:, sc * 8:], sw)
            return hsT, rqT

        fetched = {t: emit_fetch(t) for t in range(min(PF_TILES, T_TILES))}

        # ---- h_n_qr output: batched hq gather -> DRAM store
        hnq_sb = const.tile([P, T_TILES * P], f16, tag="hnq")
        if "hnq" in DISABLE:
            nc.vector.memset(hnq_sb[:], 0.0)
        for (b0, tb) in ([] if "hnq" in DISABLE else hnq_bat):
            nc.gpsimd.dma_gather(
                out_ap=hnq_sb[:, b0 * P:(b0 + tb) * P].rearrange(
                    "p (t d) -> p t d", d=P),
                in_ap=hq16[:],
                idxs_ap=hnqidx_s[:, b0 * 8:(b0 + tb) * 8],
                num_idxs=tb * P, num_idxs_reg=nidx_regs[tb * P],
                elem_size=D, transpose=False, single_packet=False)
        nc.sync.dma_start(out_hnqr[:], hnq_sb[:])

        # ---- macro pipeline stages as a generator (one PSUM bank / stream)
        tile_state = {}

        mctr = [0]

        def macro_gen(t, base_c, sc, m0, mc, first_alpha, last_of_tile):
            my_id = mctr[0]
            mctr[0] += 1
            st = tile_state[t]
            hsT, rqT, agg = st["hsT"], st["rqT"], st["agg"]
            co = int(col_off[t])
            g0 = base_c + m0                 # global chunk within tile
            ew = mc * P
            hs_sl = hsT[:, (base_c + m0) * P:(base_c + m0) * P + ew]
            hr_sl = rqT[:, (2 * base_c + m0) * P:
                        (2 * base_c + m0) * P + ew]
            hq_sl = rqT[:, (2 * base_c + sc + m0) * P:
                        (2 * base_c + sc + m0) * P + ew]

            G = p_g.tile([P, 512], f32, tag="G")
            G16 = G[:].bitcast(f16)

            mm(G[:, :ew], lhsT=wz_t_s[:], rhs=hr_sl, start=True, stop=False)
            mm(G[:, :ew], lhsT=wz_b_s[:], rhs=hq_sl, start=False, stop=False)
            mm(G[:, :ew], lhsT=uz_s[:], rhs=hs_sl, start=False, stop=True)
            yield
            z_sb = mac.tile([P, MACRO * P], f16, tag="z")
            act(z_sb[:, :ew], G[:, :ew], AF.Sigmoid, bias=bz_s[:, :1])
            yield
            mm(G[:, :ew], lhsT=wr_t_s[:], rhs=hr_sl, start=True, stop=False)
            mm(G[:, :ew], lhsT=wr_b_s[:], rhs=hq_sl, start=False, stop=False)
            mm(G[:, :ew], lhsT=ur_s[:], rhs=hs_sl, start=False, stop=True)
            yield
            r_sb = mac.tile([P, MACRO * P], f16, tag="r")
            act(r_sb[:, :ew], G[:, :ew], AF.Sigmoid, bias=br_s[:, :1])
            yield
            rh = mac.tile([P, MACRO * P], f16, tag="rh")
            nc.vector.tensor_tensor(out=rh[:, :ew], in0=r_sb[:, :ew],
                                    in1=hs_sl, op=ALU.mult)
            yield
            mm(G[:, :ew], lhsT=wh_t_s[:], rhs=hr_sl, start=True, stop=False)
            mm(G[:, :ew], lhsT=wh_b_s[:], rhs=hq_sl, start=False, stop=False)
            mm(G[:, :ew], lhsT=uh_s[:], rhs=rh[:, :ew], start=False,
               stop=True)
            yield
            ht = mac.tile([P, MACRO * P], f16, tag="ht")
            act(ht[:, :ew], G[:, :ew], AF.Tanh, bias=bh_s[:, :1])
            yield
            dd = mac.tile([P, MACRO * P], f16, tag="dd")
            nc.vector.tensor_tensor(out=dd[:, :ew], in0=ht[:, :ew],
                                    in1=hs_sl, op=ALU.subtract)
            zd = mac.tile([P, MACRO * P], f16, tag="zd")
            nc.vector.tensor_tensor(out=zd[:, :ew], in0=z_sb[:, :ew],
                                    in1=dd[:, :ew], op=ALU.mult)
            msgT = mac.tile([P, MACRO * P], f16, tag="msgT")
            nc.vector.tensor_tensor(out=msgT[:, :ew], in0=zd[:, :ew],
                                    in1=hs_sl, op=ALU.add)
            yield
            mm(G[:, :ew], lhsT=ws_s[:], rhs=msgT[:, :ew], start=True,
               stop=False)
            mm(G[:, :ew], lhsT=wqr_s[:], rhs=hq_sl, start=False, stop=True)
            yield
            relu_sb = mac.tile([P, MACRO * P], f16, tag="relu")
            if RELU_SPLIT and my_id % RELU_SPLIT == 0:
                act(relu_sb[:, :ew], G[:, :ew], AF.Relu, bias=bqr_s[:, :1])
            else:
                nc.vector.tensor_scalar(
                    out=relu_sb[:, :ew], in0=G[:, :ew],
                    scalar1=bqr_s[:, :1], scalar2=0.0,
                    op0=ALU.add, op1=ALU.max)
            yield
            for c in range(mc):
                col = 140 + g0 + c
                mm(agg[:, col:col + 1],
                   lhsT=relu_sb[:, c * P:(c + 1) * P], rhs=walpha_s[:],
                   start=(first_alpha and c == 0), stop=True,
                   skip_group_check=True)
            yield
            expc = mac.tile([P, MACRO], f32, tag="expc")
            act(expc[:, :mc], agg[:, 140 + g0:140 + g0 + mc], AF.Exp,
                bias=balpha_s[:, :1])
            yield
            pw = pwp.tile([P, MACRO * P], f16, tag="pw")
            for c in range(mc):
                nc.vector.tensor_scalar(
                    out=pw[:, c * P:(c + 1) * P], in0=iota16_s[:],
                    scalar1=obj32_s[:, co + g0 + c:co + g0 + c + 1],
                    scalar2=expc[:, c:c + 1],
                    op0=ALU.is_equal, op1=ALU.mult)
            if "msgE_T" not in DISABLE:
                for c in range(mc):
                    mm(G16[:, c * P:(c + 1) * P],
                       lhsT=msgT[:, c * P:(c + 1) * P],
                       rhs=ident16_s[:], is_transpose=True,
                       start=(c == 0), stop=(c == mc - 1))
            yield
            msgE = pwp.tile([P, MACRO * 129], f16, tag="msgE")
            mview = msgE[:].rearrange("p (c x) -> p c x", x=129)
            nc.vector.memset(mview[:, :mc, 128:129], 1.0)
            if "msgE_T" in DISABLE:
                nc.vector.memset(mview[:, :mc, 0:128], 0.5)
            else:
                nc.vector.tensor_copy(
                    mview[:, :mc, 0:128],
                    G16[:, :ew].rearrange("p (c x) -> p c x", x=P))
            yield
            for c in range(mc):
                mm(agg[:, 0:129], lhsT=pw[:, c * P:(c + 1) * P],
                   rhs=mview[:, c, 0:129],
                   start=False,
                   stop=(last_of_tile and c == mc - 1),
                   skip_group_check=True)
            if not last_of_tile:
                return
            # ---- finalize (only the tile's LAST macro reaches here, after
            # every other macro of the tile has emitted its agg matmuls)
            yield
            recip = fin.tile([P, 1], f32, tag="recip")
            nc.vector.reciprocal(recip[:], agg[:, 128:129])
            magg = fin.tile([P, P], f16, tag="magg")
            nc.vector.tensor_scalar(out=magg[:], in0=agg[:, 0:128],
                                    scalar1=recip[:, :1], scalar2=None,
                                    op0=ALU.mult)
            yield
            mm(G16[:, 512:640], lhsT=magg[:], rhs=ident16_s[:],
               is_transpose=True, start=True, stop=True,
               skip_group_check=True)
            yield
            maggT = fin.tile([P, P], f16, tag="maggT")
            nc.vector.tensor_copy(maggT[:], G16[:, 512:640])
            yield
            mm(agg[:, 160:288], lhsT=whout_s[:], rhs=maggT[:],
               start=False, stop=True, skip_group_check=True)
            yield
            hnew = fin.tile([P, P], f32, tag="hnew")
            act(hnew[:], agg[:, 160:288], AF.Relu)
            yield
            nc.sync.dma_start(out_ht[:, t * P:(t + 1) * P], hnew[:])

        # ---- job list: per tile, macros split within each slot section
        jobs = []
        for t in range(T_TILES):
            cE, cO = C_list[t]
            C2t = cE + cO
            macros = []
            for base_c, sc in ((0, cE), (cE, cO)):
                m0 = 0
                while m0 < sc:
                    macros.append((base_c, sc, m0, min(MACRO, sc - m0)))
                    m0 += MACRO
            for k, (base_c, sc, m0, mc) in enumerate(macros):
                jobs.append(("m", t, base_c, sc, m0, mc, k == 0,
                             k == len(macros) - 1))

        # ---- stream scheduler: round-robin one stage per sweep, with
        # admission staggered so streams don't hit the same engine's
        # stage in the same sweep (STAGGER sweeps of initial delay)
        from collections import deque
        pending = deque(jobs)
        active = []          # [gen, delay]
        stag = 0
        nadm = 0
        while pending or active:
            while len(active) < NSTREAM and pending:
                job = pending.popleft()
                _, t, base_c, sc, m0, mc, first, last = job
                if t not in tile_state:
                    tile_state[t] = {"agg": None}
                    hsT, rqT = fetched.pop(t)
                    tile_state[t].update(hsT=hsT, rqT=rqT)
                    if t + PF_TILES < T_TILES:
                        fetched[t + PF_TILES] = emit_fetch(t + PF_TILES)
                if first:
                    tile_state[t]["agg"] = p_ag.tile(
                        [P, 512], f32, tag="agg", name="agg")
                g = macro_gen(t, base_c, sc, m0, mc, first, last)
                active.append([g, stag])
                if nadm < NSTREAM - 1:
                    stag += STAGGER
                    nadm += 1
            stag = max(0, stag - 1)
            for ent in list(active):
                if ent[1] > 0:
                    ent[1] -= 1
                    continue
                try:
                    next(ent[0])
                except StopIteration:
                    active.remove(ent)

    return nc


# ----------------------------------------------------------------- kernel()
def kernel(hidden, rela_embed, Wz, Uz, bz, Wr_g, Ur, br, Whh, Uh, bh,
           Ws_attn, Wqr_attn, b_qr, w_alpha, b_alpha, W_h,
           q_rel, edges, n_node):
    _install_wait_splitter()

    hidden = np.asarray(hidden, np.float32)
    rela_embed = np.asarray(rela_embed, np.float32)
    edges = np.asarray(edges)
    q_rel = np.asarray(q_rel)

    meta = _host_prep(hidden, rela_embed, q_rel, edges)
    C_list, col_off, CT = meta["C_list"], meta["col_off"], meta["CT"]

    hq = rela_embed[np.asarray(q_rel, np.int64)]          # [NQ, D]
    relap = np.zeros((NREP, D), np.float32)
    relap[:NRE] = rela_embed

    nc = _build_program(C_list, col_off, CT)
    # lower InstISA subclasses (the gpsimd library-load pseudo op) to real
    # MODIFY_POOL_CONFIG encodings so walrus can compile them
    mybir.codegen_inst_isa_subclasses(nc)

    hid16 = hidden.astype(np.float16)
    common = {
        "hidE": hid16[:NSPLIT],
        "hidO": hid16[NSPLIT:],
        "relap": relap.astype(np.float16),
        "hq16": hq.astype(np.float16),
        "wz_t": np.asarray(Wz[:D], np.float16),
        "wz_b": np.asarray(Wz[D:], np.float16),
        "uz": np.asarray(Uz, np.float16),
        "wr_t": np.asarray(Wr_g[:D], np.float16),
        "wr_b": np.asarray(Wr_g[D:], np.float16),
        "ur": np.asarray(Ur, np.float16),
        "wh_t": np.asarray(Whh[:D], np.float16),
        "wh_b": np.asarray(Whh[D:], np.float16),
        "uh": np.asarray(Uh, np.float16),
        "ws": np.asarray(Ws_attn, np.float16),
        "wqr": np.asarray(Wqr_attn, np.float16),
        "walpha": np.asarray(w_alpha, np.float16).reshape(A, 1),
        "wh_out": np.asarray(W_h, np.float16),
        "bz": np.asarray(bz, np.float32).reshape(D, 1),
        "br": np.asarray(br, np.float32).reshape(D, 1),
        "bh": np.asarray(bh, np.float32).reshape(D, 1),
        "bqr": np.asarray(b_qr, np.float32).reshape(A, 1),
        "balpha": np.full((P, 1), float(np.asarray(b_alpha).reshape(-1)[0]),
                          np.float32),
        "iota16": np.broadcast_to(np.arange(P, dtype=np.float16),
                                  (P, P)).copy(),
        "ident16": np.eye(P, dtype=np.float16),
    }
    in_maps = []
    for core in range(NCORES):
        m = dict(common)
        m["obj32"] = meta["obj32"][core]
        m["hsidx"] = meta["hsidx"][core]
        m["rqidx"] = meta["rqidx"][core]
        m["hnqidx"] = meta["hnqidx"][core]
        in_maps.append(m)

    res = run_bass_kernel_spmd(nc, in_maps, list(range(NCORES))).results

    hidden_new = np.empty((N_PAD, D), np.float32)
    h_n_qr = np.empty((N_PAD, D), np.float32)
    for core in range(NCORES):
        lo = core * NODES_PER_CORE
        hi = lo + NODES_PER_CORE
        hidden_new[lo:hi] = res[core]["out_ht"].T
        h_n_qr[lo:hi] = (res[core]["out_hnqr"].astype(np.float32)
                         .reshape(P, T_TILES, P).transpose(1, 0, 2)
                         .reshape(NODES_PER_CORE, D))

    return hidden_new[:N_NODE], h_n_qr[:N_NODE]
